# revision 1
# baseline (speedup 1.0000x reference)
"""GCN neighborhood mean-aggregation kernel for Trainium2 (8 NeuronCores).

Data-parallel over the batch of target nodes: the embedding table is
replicated to every core, nodes/neigh_idx are sharded along dim 0.  Each
core gathers its nodes' 33 rows (self + 32 sampled neighbors) via SWDGE
indirect DMA and mean-reduces them on VectorE.
"""

import numpy as np

from concourse import bass, bacc, mybir
import concourse.tile as tile
from concourse.bass_utils import run_bass_kernel_spmd

V, D = 100000, 128
B, K = 50000, 32
KP1 = K + 1  # 33 rows per node: self + neighbors
NCORES = 8
P = 128
NBLK = 49            # node blocks of 128 per core
BLOC = NBLK * P      # 6272 padded nodes per core
BPAD = BLOC * NCORES # 50176 >= B


def _build(nblk: int) -> bass.Bass:
    # idx DRAM layout is partition-major: idx[p, i*KP1 + k] holds the k-th
    # index of node i*128+p — the one-shot preload below is then a single
    # contiguous [128, nblk*KP1] DMA, and each block's offsets are a
    # contiguous per-partition slice of the persistent SBUF buffer.
    nc = bacc.Bacc(None)
    feats = nc.declare_dram_parameter(
        "features", [V, D], mybir.dt.float32, isOutput=False
    )
    idx = nc.declare_dram_parameter(
        "idx", [P, nblk * KP1], mybir.dt.int32, isOutput=False
    )
    out = nc.declare_dram_parameter(
        "out", [nblk * P, D], mybir.dt.float32, isOutput=True
    )

    with tile.TileContext(nc) as tc:
        with (
            tc.tile_pool(name="const", bufs=1) as cpool,
            tc.tile_pool(name="sbuf", bufs=3) as pool,
        ):
            idx_buf = cpool.tile([P, nblk * KP1], mybir.dt.int32)
            nc.sync.dma_start(out=idx_buf[:], in_=idx[:])
            for i in range(nblk):
                # 33 gathers per block: the HW indirect DMA consumes ONE
                # offset per partition per instruction, so gather k fetches
                # feats[idx[p, k]] into partition p's k-th row slot.
                gath = pool.tile([P, KP1 * D], mybir.dt.float32, tag="gath")
                for k in range(KP1):
                    nc.gpsimd.indirect_dma_start(
                        out=gath[:, k * D : (k + 1) * D],
                        out_offset=None,
                        in_=feats[:],
                        in_offset=bass.IndirectOffsetOnAxis(
                            ap=idx_buf[:, i * KP1 + k : i * KP1 + k + 1], axis=0
                        ),
                    )
                # Tree-reduce 33 rows into row block 0: fold row 32 in, then
                # halve 32 -> 16 -> 8 -> 4 -> 2 -> 1.
                nc.vector.tensor_add(
                    out=gath[:, 0:D], in0=gath[:, 0:D], in1=gath[:, 32 * D : 33 * D]
                )
                w = 16 * D
                while w >= D:
                    nc.vector.tensor_add(
                        out=gath[:, 0:w], in0=gath[:, 0:w], in1=gath[:, w : 2 * w]
                    )
                    w //= 2
                ot = pool.tile([P, D], mybir.dt.float32, tag="ot")
                nc.vector.tensor_scalar_mul(ot[:], gath[:, 0:D], 1.0 / KP1)
                nc.sync.dma_start(out=out[i * P : (i + 1) * P, :], in_=ot[:])
    nc.finalize()
    return nc


_CACHE: dict = {}

# test-harness knobs (the grading harness leaves these at defaults)
TRACE = False
LAST_RESULTS = None


def _get_nc() -> bass.Bass:
    if "nc" not in _CACHE:
        _CACHE["nc"] = _build(NBLK)
    return _CACHE["nc"]


def kernel(features, nodes, neigh_idx):
    feats = np.ascontiguousarray(np.asarray(features), dtype=np.float32)
    nodes = np.asarray(nodes)
    neigh = np.asarray(neigh_idx)

    idx_all = np.zeros((BPAD, KP1), dtype=np.int32)
    idx_all[:B, 0] = nodes.astype(np.int32)
    idx_all[:B, 1:] = neigh.astype(np.int32)
    # per-core partition-major layout: [NBLK, P, KP1] -> [P, NBLK*KP1]
    shards = (
        idx_all.reshape(NCORES, NBLK, P, KP1)
        .transpose(0, 2, 1, 3)
        .reshape(NCORES, P, NBLK * KP1)
    )

    nc = _get_nc()
    in_maps = [
        {"features": feats, "idx": np.ascontiguousarray(shards[c])}
        for c in range(NCORES)
    ]
    res = run_bass_kernel_spmd(nc, in_maps, list(range(NCORES)), trace=TRACE)
    global LAST_RESULTS
    LAST_RESULTS = res
    out = np.concatenate([res.results[c]["out"] for c in range(NCORES)], axis=0)
    return out[:B]



# revision 12
# speedup vs baseline: 3.7467x; 3.7467x over previous
"""GCN neighborhood mean-aggregation kernel for Trainium2 (8 NeuronCores).

Data-parallel over target nodes (6400/core).  The f32 feature table is
converted to bf16 on the host and replicated; each core gathers its
nodes' 33 rows (self + 32 neighbors) with multi-descriptor SWDGE
dma_gather instructions spread over 4 SWDGE queues, then reduces with
PE matmuls against on-chip-generated one-hot match matrices (PSUM
accumulation), scales by 1/33 and stores f32.

dma_gather indices are int16, so the 100000-row table is addressed as 4
value buckets of <=32768 rows; each node-group's slots are bucket-major
sorted on the host.  The slot->node map (scrambled by the bucket sort)
ships as a per-slot node-id byte which is expanded on-chip into 0/1
match matrices via is_equal against an iota row.
"""

import numpy as np
import ml_dtypes

from concourse import bass, bacc, mybir
import concourse.tile as tile
from concourse.bass_utils import run_bass_kernel_spmd

V, D = 100000, 128
B, K = 50000, 32
KP1 = K + 1
NCORES = 8
P = 128

NODES_PC = 6400            # nodes per core (padded)
NGRP = NODES_PC // P       # 50 groups of 128 nodes
NSUP = NGRP // 2           # 25 supers of 2 groups
BPAD = NODES_PC * NCORES   # 51200 >= B

NBKT = 4
BKT_BASE = [0, 32768, 65536, 98304]
BKT_ROWS = [32768, 32768, 32768, V - 98304]

MAX_IDXS = 1024            # per-instruction SWDGE ring-capacity limit
NQ = 4                     # SWDGE queues

PAD_NODEREL = 255.0


def _chunks(total: int) -> list[int]:
    """Split a slot count (multiple of 128) into <=MAX_IDXS pieces."""
    out = [MAX_IDXS] * (total // MAX_IDXS)
    if total % MAX_IDXS:
        out.append(total % MAX_IDXS)
    return out


def _build(sb: tuple[int, ...]) -> bass.Bass:
    """sb: per-bucket padded slot budgets per 128-node group (each %128==0)."""
    nbg = [s // P for s in sb]        # blocks per (group, bucket)
    nblk_g = sum(nbg)                 # blocks per group
    nblk_sup = 2 * nblk_g             # blocks per super tile
    sg = sum(sb)                      # slots per group
    slots_sup = 2 * sg
    w_sup = slots_sup // 16           # idx columns per super
    maxnb = max(nbg)

    nc = bacc.Bacc(None, num_swdge_queues=NQ)
    feats = nc.declare_dram_parameter(
        "features", [V, D], mybir.dt.bfloat16, isOutput=False
    )
    idx16 = nc.declare_dram_parameter(
        "idx16", [P, NSUP * w_sup], mybir.dt.int16, isOutput=False
    )
    nodrel = nc.declare_dram_parameter(
        "nodrel", [P, NSUP * nblk_sup], mybir.dt.bfloat16, isOutput=False
    )
    iota = nc.declare_dram_parameter(
        "iota", [P, maxnb * P], mybir.dt.bfloat16, isOutput=False
    )
    out = nc.declare_dram_parameter(
        "out", [NODES_PC, D], mybir.dt.float32, isOutput=True
    )

    # per-(parity, bucket) block offset of a group's bucket-b run inside the
    # super tile: buckets are laid out [b0:gA,gB][b1:gA,gB]...
    run_off = {}
    acc = 0
    for b in range(NBKT):
        for pi in range(2):
            run_off[(pi, b)] = acc + pi * nbg[b]
        acc += 2 * nbg[b]

    qctr = 0
    with tile.TileContext(nc) as tc:
        with (
            tc.tile_pool(name="const", bufs=1) as cpool,
            tc.tile_pool(name="sbuf", bufs=3) as pool,
            tc.tile_pool(name="psum", bufs=2, space="PSUM") as ppool,
        ):
            # split the idx preload so the first supers' gathers aren't gated
            # on the full 3.8MB transfer (subtile deps release per chunk)
            idx_buf = cpool.tile([P, NSUP * w_sup], mybir.dt.int16)
            pre = [1, 4, 10, NSUP]
            for lo, hi in zip([0] + pre[:-1], pre):
                nc.sync.dma_start(
                    out=idx_buf[:, lo * w_sup : hi * w_sup],
                    in_=idx16[:, lo * w_sup : hi * w_sup],
                )
            nr_buf = cpool.tile([P, NSUP * nblk_sup], mybir.dt.bfloat16)
            nc.sync.dma_start(out=nr_buf[:], in_=nodrel[:])
            io_buf = cpool.tile([P, maxnb * P], mybir.dt.bfloat16)
            nc.sync.dma_start(out=io_buf[:], in_=iota[:])

            for s in range(NSUP):
                gath = pool.tile([P, nblk_sup, D], mybir.dt.bfloat16, tag="gath")
                # 4 value buckets x chunks of <=1024 idxs, round-robin queues
                blk0 = 0
                col0 = s * w_sup
                for b in range(NBKT):
                    off = 0
                    for n in _chunks(2 * sb[b]):
                        nc.gpsimd.dma_gather(
                            out_ap=gath[
                                :, blk0 + off // P : blk0 + (off + n) // P, :
                            ],
                            in_ap=feats[BKT_BASE[b] : BKT_BASE[b] + BKT_ROWS[b]],
                            idxs_ap=idx_buf[
                                :, col0 + off // 16 : col0 + (off + n) // 16
                            ],
                            num_idxs=n,
                            num_idxs_reg=n,
                            elem_size=D,
                            queue_num=qctr % NQ,
                        )
                        qctr += 1
                        off += n
                    blk0 += 2 * nbg[b]
                    col0 += 2 * sb[b] // 16

                for pi in range(2):
                    g = 2 * s + pi
                    m = pool.tile([P, nblk_g * P], mybir.dt.bfloat16, tag="m")
                    # match matrix per bucket run: m[p, j, n] = (nodrel == n)
                    moff = 0
                    for b in range(NBKT):
                        nb = nbg[b]
                        c0 = s * nblk_sup + run_off[(pi, b)]
                        nc.vector.tensor_tensor(
                            out=m[:, moff : moff + nb * P].rearrange(
                                "p (j n) -> p j n", j=nb, n=P
                            ),
                            in0=nr_buf[:, c0 : c0 + nb].to_broadcast([P, nb, P]),
                            in1=io_buf[:, : nb * P].rearrange(
                                "p (j n) -> p j n", j=nb, n=P
                            ),
                            op=mybir.AluOpType.is_equal,
                        )
                        moff += nb * P
                    ps = ppool.tile([P, D], mybir.dt.float32, tag="ps")
                    j = 0
                    for b in range(NBKT):
                        for r in range(nbg[b]):
                            nc.tensor.matmul(
                                out=ps[:],
                                lhsT=m[:, j * P : (j + 1) * P],
                                rhs=gath[:, run_off[(pi, b)] + r, :],
                                start=(j == 0),
                                stop=(j == nblk_g - 1),
                            )
                            j += 1
                    ot = pool.tile([P, D], mybir.dt.float32, tag="ot")
                    nc.vector.tensor_scalar_mul(ot[:], ps[:], 1.0 / KP1)
                    nc.sync.dma_start(
                        out=out[g * P : (g + 1) * P, :], in_=ot[:]
                    )
    nc.finalize()
    return nc


def _wrap16(lists: np.ndarray) -> np.ndarray:
    """[..., n] int16 -> [..., 128, n//16]: pos j -> (partition j%16, col j//16),
    replicated to all 8 partition groups."""
    *lead, n = lists.shape
    w = lists.reshape(*lead, n // 16, 16)
    w = np.moveaxis(w, -1, -2)  # [..., 16, n//16]
    return np.tile(w, (*([1] * len(lead)), 8, 1))


def _prep(nodes: np.ndarray, neigh: np.ndarray):
    """Bucket-sort slots per (core, group); returns per-core host arrays."""
    idx_all = np.zeros((BPAD, KP1), dtype=np.int32)
    idx_all[:B, 0] = nodes.astype(np.int32)
    idx_all[:B, 1:] = neigh.astype(np.int32)
    real = np.zeros((BPAD,), dtype=bool)
    real[:B] = True

    # [NCORES, NGRP, 128*KP1] node-major slots
    slots = idx_all.reshape(NCORES, NGRP, P, KP1).reshape(NCORES, NGRP, P * KP1)
    srel = np.broadcast_to(
        (np.arange(P * KP1) // KP1)[None, None, :], slots.shape
    )
    sreal = real.reshape(NCORES, NGRP, P, 1)
    sreal = np.broadcast_to(sreal, (NCORES, NGRP, P, KP1)).reshape(
        NCORES, NGRP, P * KP1
    )

    bkt = (slots >> 15).astype(np.int8)
    bkt = np.where(sreal, bkt, np.int8(NBKT))  # pad-node slots -> dropped
    order = np.argsort(bkt, axis=-1, kind="stable")
    s_idx = np.take_along_axis(slots, order, axis=-1)
    s_rel = np.take_along_axis(srel, order, axis=-1)
    s_bkt = np.take_along_axis(bkt, order, axis=-1)

    counts = (s_bkt[..., None] == np.arange(NBKT)).sum(axis=2)  # [C, G, NBKT]
    sb = counts.max(axis=(0, 1))
    sb = np.maximum(128, ((sb + 127) // 128) * 128)  # budgets, %128
    sg = int(sb.sum())

    # scatter sorted slots into padded per-group layout
    starts = np.concatenate(
        [np.zeros_like(counts[..., :1]), np.cumsum(counts, axis=-1)[..., :-1]],
        axis=-1,
    )  # start of each bucket in sorted order
    bases = np.concatenate([[0], np.cumsum(sb)[:-1]])  # padded bucket bases

    nslots = P * KP1
    pos_in_bucket = np.arange(nslots)[None, None, :] - np.take_along_axis(
        starts, s_bkt.clip(max=NBKT - 1).astype(np.int64), axis=-1
    )
    dst = bases[s_bkt.clip(max=NBKT - 1)] + pos_in_bucket
    keep = s_bkt < NBKT

    pad_idx = np.zeros((NCORES, NGRP, sg), dtype=np.int32)
    pad_rel = np.full((NCORES, NGRP, sg), 255, dtype=np.int32)
    ci, gi, _ = np.meshgrid(
        np.arange(NCORES), np.arange(NGRP), np.arange(nslots), indexing="ij"
    )
    rebase = np.array(BKT_BASE, dtype=np.int32)[
        s_bkt.clip(max=NBKT - 1).astype(np.int64)
    ]
    pad_idx[ci[keep], gi[keep], dst[keep]] = (s_idx - rebase)[keep]
    pad_rel[ci[keep], gi[keep], dst[keep]] = s_rel[keep]

    # super layout: [b0: gA | gB][b1: gA | gB]...
    pad_idx = pad_idx.reshape(NCORES, NSUP, 2, sg)
    pad_rel = pad_rel.reshape(NCORES, NSUP, 2, sg)
    seg_i, seg_r = [], []
    for b in range(NBKT):
        sl = slice(int(bases[b]), int(bases[b] + sb[b]))
        seg_i.append(pad_idx[:, :, :, sl].reshape(NCORES, NSUP, 2 * int(sb[b])))
        seg_r.append(pad_rel[:, :, :, sl].reshape(NCORES, NSUP, 2 * int(sb[b])))
    sup_idx = np.concatenate(seg_i, axis=2)  # [C, NSUP, 2*sg]
    sup_rel = np.concatenate(seg_r, axis=2)

    idx16 = _wrap16(sup_idx.astype(np.int16))  # [C, NSUP, 128, w_sup]
    idx16 = idx16.transpose(0, 2, 1, 3).reshape(NCORES, P, -1)

    nblk_sup = 2 * sg // P
    nrel = sup_rel.reshape(NCORES, NSUP, nblk_sup, P)
    nrel = nrel.transpose(0, 3, 1, 2).reshape(NCORES, P, NSUP * nblk_sup)
    nodrel = nrel.astype(ml_dtypes.bfloat16)

    maxnb = int(sb.max()) // P
    iota = np.tile(np.arange(P, dtype=np.float32), maxnb).astype(ml_dtypes.bfloat16)
    iota = np.broadcast_to(iota[None, :], (P, maxnb * P)).copy()

    return tuple(int(x) for x in sb), idx16, nodrel, iota


_CACHE: dict = {}

# test-harness knobs (the grading harness leaves these at defaults)
TRACE = False
LAST_RESULTS = None


def kernel(features, nodes, neigh_idx):
    feats = np.asarray(features).astype(ml_dtypes.bfloat16)
    nodes = np.asarray(nodes)
    neigh = np.asarray(neigh_idx)

    sb, idx16, nodrel, iota = _prep(nodes, neigh)
    if ("nc", sb) not in _CACHE:
        _CACHE[("nc", sb)] = _build(sb)
    nc = _CACHE[("nc", sb)]

    in_maps = [
        {
            "features": feats,
            "idx16": np.ascontiguousarray(idx16[c]),
            "nodrel": np.ascontiguousarray(nodrel[c]),
            "iota": iota,
        }
        for c in range(NCORES)
    ]
    res = run_bass_kernel_spmd(nc, in_maps, list(range(NCORES)), trace=TRACE)
    global LAST_RESULTS
    LAST_RESULTS = res
    out = np.concatenate([res.results[c]["out"] for c in range(NCORES)], axis=0)
    return out[:B]


# revision 15
# speedup vs baseline: 4.1788x; 1.1153x over previous
"""GCN neighborhood mean-aggregation kernel for Trainium2 (8 NeuronCores).

Data-parallel over target nodes (6400/core).  The f32 feature table is
converted to bf16 on the host and replicated; each core gathers its
nodes' 33 rows (self + 32 neighbors) with multi-descriptor SWDGE
dma_gather instructions spread over 4 SWDGE queues, then reduces with
PE matmuls against on-chip-generated one-hot match matrices (PSUM
accumulation), scales by 1/33 and stores f32.

dma_gather indices are int16, so the 100000-row table is addressed as 4
value buckets of <=32768 rows; each node-group's slots are bucket-major
sorted on the host.  The slot->node map (scrambled by the bucket sort)
ships as a per-slot node-id byte which is expanded on-chip into 0/1
match matrices via is_equal against an iota row.
"""

import numpy as np
import ml_dtypes

from concourse import bass, bacc, mybir
import concourse.tile as tile
from concourse.bass_utils import run_bass_kernel_spmd

V, D = 100000, 128
B, K = 50000, 32
KP1 = K + 1
NCORES = 8
P = 128

NODES_PC = 6400            # nodes per core (padded)
NGRP = NODES_PC // P       # 50 groups of 128 nodes
NSUP = NGRP // 2           # 25 supers of 2 groups
BPAD = NODES_PC * NCORES   # 51200 >= B

NBKT = 4
BKT_BASE = [0, 32768, 65536, 98304]
BKT_ROWS = [32768, 32768, 32768, V - 98304]

MAX_IDXS = 1024            # per-instruction SWDGE ring-capacity limit
NQ = 4                     # SWDGE queues

PAD_NODEREL = 255.0


def _chunks(total: int) -> list[int]:
    """Split a slot count (multiple of 128) into <=MAX_IDXS pieces."""
    out = [MAX_IDXS] * (total // MAX_IDXS)
    if total % MAX_IDXS:
        out.append(total % MAX_IDXS)
    return out


def _build(sb: tuple[int, ...]) -> bass.Bass:
    """sb: per-bucket padded slot budgets per 128-node group (each %128==0)."""
    nbg = [s // P for s in sb]        # blocks per (group, bucket)
    nblk_g = sum(nbg)                 # blocks per group
    nblk_sup = 2 * nblk_g             # blocks per super tile
    sg = sum(sb)                      # slots per group
    slots_sup = 2 * sg
    w_sup = slots_sup // 16           # idx columns per super
    maxnb = max(nbg)

    nc = bacc.Bacc(None, num_swdge_queues=NQ)
    feats = nc.declare_dram_parameter(
        "features", [V, D], mybir.dt.bfloat16, isOutput=False
    )
    idx16 = nc.declare_dram_parameter(
        "idx16", [P, NSUP * w_sup], mybir.dt.int16, isOutput=False
    )
    nodrel = nc.declare_dram_parameter(
        "nodrel", [P, NSUP * nblk_sup], mybir.dt.bfloat16, isOutput=False
    )
    iota = nc.declare_dram_parameter(
        "iota", [P, maxnb * P], mybir.dt.bfloat16, isOutput=False
    )
    out = nc.declare_dram_parameter(
        "out", [NODES_PC, D], mybir.dt.float32, isOutput=True
    )

    # per-(parity, bucket) block offset of a group's bucket-b run inside the
    # super tile: buckets are laid out [b0:gA,gB][b1:gA,gB]...
    run_off = {}
    acc = 0
    for b in range(NBKT):
        for pi in range(2):
            run_off[(pi, b)] = acc + pi * nbg[b]
        acc += 2 * nbg[b]

    qctr = 0
    with tile.TileContext(nc) as tc:
        with (
            tc.tile_pool(name="const", bufs=1) as cpool,
            tc.tile_pool(name="sbuf", bufs=3) as pool,
            tc.tile_pool(name="psum", bufs=2, space="PSUM") as ppool,
        ):
            # split the idx preload so the first supers' gathers aren't gated
            # on the full 3.8MB transfer (subtile deps release per chunk)
            idx_buf = cpool.tile([P, NSUP * w_sup], mybir.dt.int16)
            pre = [1, 4, 10, NSUP]
            for lo, hi in zip([0] + pre[:-1], pre):
                nc.sync.dma_start(
                    out=idx_buf[:, lo * w_sup : hi * w_sup],
                    in_=idx16[:, lo * w_sup : hi * w_sup],
                )
            nr_buf = cpool.tile([P, NSUP * nblk_sup], mybir.dt.bfloat16)
            nc.sync.dma_start(out=nr_buf[:], in_=nodrel[:])
            io_buf = cpool.tile([P, maxnb * P], mybir.dt.bfloat16)
            nc.sync.dma_start(out=io_buf[:], in_=iota[:])

            for s in range(NSUP):
                gath = pool.tile([P, nblk_sup, D], mybir.dt.bfloat16, tag="gath")
                # 4 value buckets x chunks of <=1024 idxs, round-robin queues
                blk0 = 0
                col0 = s * w_sup
                for b in range(NBKT):
                    off = 0
                    for n in _chunks(2 * sb[b]):
                        nc.gpsimd.dma_gather(
                            out_ap=gath[
                                :, blk0 + off // P : blk0 + (off + n) // P, :
                            ],
                            in_ap=feats[BKT_BASE[b] : BKT_BASE[b] + BKT_ROWS[b]],
                            idxs_ap=idx_buf[
                                :, col0 + off // 16 : col0 + (off + n) // 16
                            ],
                            num_idxs=n,
                            num_idxs_reg=n,
                            elem_size=D,
                            queue_num=qctr % NQ,
                        )
                        qctr += 1
                        off += n
                    blk0 += 2 * nbg[b]
                    col0 += 2 * sb[b] // 16

                for pi in range(2):
                    g = 2 * s + pi
                    m = pool.tile([P, nblk_g * P], mybir.dt.bfloat16, tag="m")
                    # match matrix per bucket run: m[p, j, n] = (nodrel == n)
                    moff = 0
                    for b in range(NBKT):
                        nb = nbg[b]
                        c0 = s * nblk_sup + run_off[(pi, b)]
                        nc.vector.tensor_tensor(
                            out=m[:, moff : moff + nb * P].rearrange(
                                "p (j n) -> p j n", j=nb, n=P
                            ),
                            in0=nr_buf[:, c0 : c0 + nb].to_broadcast([P, nb, P]),
                            in1=io_buf[:, : nb * P].rearrange(
                                "p (j n) -> p j n", j=nb, n=P
                            ),
                            op=mybir.AluOpType.is_equal,
                        )
                        moff += nb * P
                    ps = ppool.tile([P, D], mybir.dt.float32, tag="ps")
                    j = 0
                    for b in range(NBKT):
                        for r in range(nbg[b]):
                            nc.tensor.matmul(
                                out=ps[:],
                                lhsT=m[:, j * P : (j + 1) * P],
                                rhs=gath[:, run_off[(pi, b)] + r, :],
                                start=(j == 0),
                                stop=(j == nblk_g - 1),
                            )
                            j += 1
                    ot = pool.tile([P, D], mybir.dt.float32, tag="ot")
                    nc.vector.tensor_scalar_mul(ot[:], ps[:], 1.0 / KP1)
                    nc.sync.dma_start(
                        out=out[g * P : (g + 1) * P, :], in_=ot[:]
                    )
    nc.finalize()
    return nc


def _wrap16(lists: np.ndarray) -> np.ndarray:
    """[..., n] int16 -> [..., 128, n//16]: pos j -> (partition j%16, col j//16),
    replicated to all 8 partition groups."""
    *lead, n = lists.shape
    w = lists.reshape(*lead, n // 16, 16)
    w = np.moveaxis(w, -1, -2)  # [..., 16, n//16]
    return np.tile(w, (*([1] * len(lead)), 8, 1))


def _balance_pos(idx_real: np.ndarray) -> np.ndarray:
    """node_at_pos[p] = original node id at device position p (-1 = pad).

    Assigns 125 real nodes + 3 pad slots to each of the 400 device groups,
    flattening per-group bucket-count maxima (budgets are max-based, so
    balancing cuts pad indices).  2D stratification: sort by bucket-0 count
    into strata of one-node-per-group, rotate bucket-1 ranks across strata.
    """
    ngroups = NCORES * NGRP  # 400
    c = (idx_real >> 15).astype(np.int64)  # [B, KP1] bucket ids
    c0 = (c == 0).sum(axis=1)
    c1 = (c == 1).sum(axis=1)
    nstrata = B // ngroups  # 125
    order0 = np.argsort(c0, kind="stable")
    strata = order0.reshape(nstrata, ngroups)
    within = np.take_along_axis(strata, np.argsort(c1[strata], axis=1), axis=1)
    rot = (np.arange(ngroups)[None, :] + 17 * np.arange(nstrata)[:, None]) % ngroups
    assign = np.empty((nstrata, ngroups), dtype=np.int64)
    np.put_along_axis(assign, rot, within, axis=1)
    arr = np.full((ngroups, P), -1, dtype=np.int64)
    arr[:, :nstrata] = assign.T  # group g rows: 125 real + 3 pads
    return arr.reshape(-1)  # [BPAD]


def _prep(nodes: np.ndarray, neigh: np.ndarray):
    """Bucket-sort slots per (core, group); returns per-core host arrays."""
    idx_real = np.concatenate(
        [nodes.astype(np.int32)[:, None], neigh.astype(np.int32)], axis=1
    )
    node_at_pos = _balance_pos(idx_real)  # [BPAD]
    real = node_at_pos >= 0
    idx_all = np.zeros((BPAD, KP1), dtype=np.int32)
    idx_all[real] = idx_real[node_at_pos[real]]

    # [NCORES, NGRP, 128*KP1] node-major slots
    slots = idx_all.reshape(NCORES, NGRP, P, KP1).reshape(NCORES, NGRP, P * KP1)
    srel = np.broadcast_to(
        (np.arange(P * KP1) // KP1)[None, None, :], slots.shape
    )
    sreal = real.reshape(NCORES, NGRP, P, 1)
    sreal = np.broadcast_to(sreal, (NCORES, NGRP, P, KP1)).reshape(
        NCORES, NGRP, P * KP1
    )

    bkt = (slots >> 15).astype(np.int8)
    bkt = np.where(sreal, bkt, np.int8(NBKT))  # pad-node slots -> dropped
    order = np.argsort(bkt, axis=-1, kind="stable")
    s_idx = np.take_along_axis(slots, order, axis=-1)
    s_rel = np.take_along_axis(srel, order, axis=-1)
    s_bkt = np.take_along_axis(bkt, order, axis=-1)

    counts = (s_bkt[..., None] == np.arange(NBKT)).sum(axis=2)  # [C, G, NBKT]
    sb = counts.max(axis=(0, 1))
    sb = np.maximum(128, ((sb + 127) // 128) * 128)  # budgets, %128
    sg = int(sb.sum())

    # scatter sorted slots into padded per-group layout
    starts = np.concatenate(
        [np.zeros_like(counts[..., :1]), np.cumsum(counts, axis=-1)[..., :-1]],
        axis=-1,
    )  # start of each bucket in sorted order
    bases = np.concatenate([[0], np.cumsum(sb)[:-1]])  # padded bucket bases

    nslots = P * KP1
    pos_in_bucket = np.arange(nslots)[None, None, :] - np.take_along_axis(
        starts, s_bkt.clip(max=NBKT - 1).astype(np.int64), axis=-1
    )
    dst = bases[s_bkt.clip(max=NBKT - 1)] + pos_in_bucket
    keep = s_bkt < NBKT

    pad_idx = np.zeros((NCORES, NGRP, sg), dtype=np.int32)
    pad_rel = np.full((NCORES, NGRP, sg), 255, dtype=np.int32)
    ci, gi, _ = np.meshgrid(
        np.arange(NCORES), np.arange(NGRP), np.arange(nslots), indexing="ij"
    )
    rebase = np.array(BKT_BASE, dtype=np.int32)[
        s_bkt.clip(max=NBKT - 1).astype(np.int64)
    ]
    pad_idx[ci[keep], gi[keep], dst[keep]] = (s_idx - rebase)[keep]
    pad_rel[ci[keep], gi[keep], dst[keep]] = s_rel[keep]

    # super layout: [b0: gA | gB][b1: gA | gB]...
    pad_idx = pad_idx.reshape(NCORES, NSUP, 2, sg)
    pad_rel = pad_rel.reshape(NCORES, NSUP, 2, sg)
    seg_i, seg_r = [], []
    for b in range(NBKT):
        sl = slice(int(bases[b]), int(bases[b] + sb[b]))
        seg_i.append(pad_idx[:, :, :, sl].reshape(NCORES, NSUP, 2 * int(sb[b])))
        seg_r.append(pad_rel[:, :, :, sl].reshape(NCORES, NSUP, 2 * int(sb[b])))
    sup_idx = np.concatenate(seg_i, axis=2)  # [C, NSUP, 2*sg]
    sup_rel = np.concatenate(seg_r, axis=2)

    idx16 = _wrap16(sup_idx.astype(np.int16))  # [C, NSUP, 128, w_sup]
    idx16 = idx16.transpose(0, 2, 1, 3).reshape(NCORES, P, -1)

    nblk_sup = 2 * sg // P
    nrel = sup_rel.reshape(NCORES, NSUP, nblk_sup, P)
    nrel = nrel.transpose(0, 3, 1, 2).reshape(NCORES, P, NSUP * nblk_sup)
    nodrel = nrel.astype(ml_dtypes.bfloat16)

    maxnb = int(sb.max()) // P
    iota = np.tile(np.arange(P, dtype=np.float32), maxnb).astype(ml_dtypes.bfloat16)
    iota = np.broadcast_to(iota[None, :], (P, maxnb * P)).copy()

    return tuple(int(x) for x in sb), idx16, nodrel, iota, node_at_pos


_CACHE: dict = {}

# test-harness knobs (the grading harness leaves these at defaults)
TRACE = False
LAST_RESULTS = None


def kernel(features, nodes, neigh_idx):
    feats = np.asarray(features).astype(ml_dtypes.bfloat16)
    nodes = np.asarray(nodes)
    neigh = np.asarray(neigh_idx)

    sb, idx16, nodrel, iota, node_at_pos = _prep(nodes, neigh)
    if ("nc", sb) not in _CACHE:
        _CACHE[("nc", sb)] = _build(sb)
    nc = _CACHE[("nc", sb)]

    in_maps = [
        {
            "features": feats,
            "idx16": np.ascontiguousarray(idx16[c]),
            "nodrel": np.ascontiguousarray(nodrel[c]),
            "iota": iota,
        }
        for c in range(NCORES)
    ]
    res = run_bass_kernel_spmd(nc, in_maps, list(range(NCORES)), trace=TRACE)
    global LAST_RESULTS
    LAST_RESULTS = res
    out = np.concatenate([res.results[c]["out"] for c in range(NCORES)], axis=0)
    valid = node_at_pos >= 0
    final = np.empty((B, D), dtype=np.float32)
    final[node_at_pos[valid]] = out[valid]
    return final
